# revision 3
# baseline (speedup 1.0000x reference)
"""Compressed (block-stride) attention on 8 Trainium2 NeuronCores.

Shards the 32 (batch, head) pairs across 8 cores (4 per core); k/v and the
block-stride mask are small and handled per-shard, so there is no cross-core
communication. Per (b,h) unit the kernel computes, in 128-row query tiles:

    s  = qT.T @ kT + mask        (PE, fp32r, accumulated in PSUM)
    p~ = exp(scale*s)            (ACT, with per-row sum accumulator)
    p  = p~ * (1/sum)            (DVE)
    o  = p @ v                   (PE transposes p, then fp32 matmul)

q and k are pre-transposed on the host (d-major) so the contraction dim lands
on SBUF partitions with clean 512B+ DMA descriptors; the additive mask is
folded into the QK PSUM accumulation as an identity-weighted matmul. The
block-stride structure makes columns >= 8t+7 of tile t fully masked, so their
(exactly zero) p values are never written -- the output buffer is pre-zeroed.
"""

import os
import sys

import numpy as np

for _p in ("/opt/trn_rl_repo", "/root/.axon_site/_ro/trn_rl_repo"):
    if os.path.isdir(_p) and _p not in sys.path:
        sys.path.insert(0, _p)

import concourse.bacc as bacc
import concourse.mybir as mybir
import concourse.tile as tile
from concourse.bass_utils import run_bass_kernel_spmd

B, M, QH, D = 2, 4096, 16, 192
N, VD = 255, 128
NP = 256                      # n padded to 256 (fp32r matmul wants free >= 256)
NCORES = 8
U = (B * QH) // NCORES        # 4 (b,h) units per core
MT = M // 128                 # 32 query tiles per unit
DC = 96                       # contraction chunk: 192 = 2 x 96
KSZ, STR = 32, 16
SCALE = float(D) ** -0.5
# power of two: scale*MASKVAL is exactly fp32-representable, so the t=0
# max-subtraction cancels exactly even in the ACT engine's extended-precision
# multiply-add (exp(0)=1 for fully-masked rows -> uniform 1/255, as reference)
MASKVAL = -(2.0 ** 100)
F32 = mybir.dt.float32
F32R = mybir.dt.float32r
AX = mybir.AxisListType.X
EXP = mybir.ActivationFunctionType.Exp

_cache: dict = {}


def _nhi(t: int) -> int:
    # first fully-masked column of tile t is 8t+7 (i < 16j+31 for all i in tile)
    return min(N, 8 * t + 7)


def _build():
    nc = bacc.Bacc()
    qT = nc.dram_tensor("qT", [U, D, M], F32R, kind="ExternalInput")
    kT = nc.dram_tensor("kT", [U, D, NP], F32R, kind="ExternalInput")
    vv = nc.dram_tensor("v", [U, N, VD], F32, kind="ExternalInput")
    mk = nc.dram_tensor("mask", [MT, 128, NP], F32R, kind="ExternalInput")
    idr = nc.dram_tensor("identr", [128, 128], F32R, kind="ExternalInput")
    id32 = nc.dram_tensor("ident32", [128, 128], F32, kind="ExternalInput")
    od = nc.dram_tensor("o", [U, M, VD], F32, kind="ExternalOutput")
    pd = nc.dram_tensor("p", [U, M, N], F32, kind="ExternalOutput")

    with tile.TileContext(nc) as tc:
        with tc.tile_pool(name="const", bufs=1) as cpool, \
             tc.tile_pool(name="kv", bufs=2) as kvpool, \
             tc.tile_pool(name="qp", bufs=4) as qpool, \
             tc.tile_pool(name="pp", bufs=3) as ppool, \
             tc.tile_pool(name="st", bufs=4) as stpool, \
             tc.tile_pool(name="ps_s", bufs=2, space="PSUM") as ps_s, \
             tc.tile_pool(name="ps_pt", bufs=2, space="PSUM") as ps_pt, \
             tc.tile_pool(name="ps_o", bufs=3, space="PSUM") as ps_o:

            identr = cpool.tile([128, 128], F32R, tag="identr")
            nc.sync.dma_start(out=identr, in_=idr[:, :])
            ident32 = cpool.tile([128, 128], F32, tag="ident32")
            nc.sync.dma_start(out=ident32, in_=id32[:, :])
            mask_sb = cpool.tile([128, MT, NP], F32R, tag="mask")
            for t in range(MT):
                nc.sync.dma_start(out=mask_sb[:, t, :], in_=mk[t])

            for u in range(U):
                kT_sb = kvpool.tile([DC, 2, NP], F32R, tag="kT")
                nc.sync.dma_start(
                    out=kT_sb, in_=kT[u].rearrange("(c p) n -> p c n", p=DC)
                )
                v_sb = kvpool.tile([128, 2, VD], F32, tag="v")
                nc.sync.dma_start(out=v_sb[:, 0, :], in_=vv[u, 0:128, :])
                nc.sync.dma_start(out=v_sb[0:127, 1, :], in_=vv[u, 128:255, :])

                for t in range(MT):
                    m0 = t * 128
                    nhi = _nhi(t)
                    nchunks = 1 if nhi <= 128 else 2

                    qT_sb = qpool.tile([DC, 2, 128], F32R, tag="q")
                    nc.sync.dma_start(
                        out=qT_sb,
                        in_=qT[u, :, m0:m0 + 128].rearrange("(c p) m -> p c m", p=DC),
                    )

                    s_ps = ps_s.tile([128, NP], F32, tag="s")
                    nc.tensor.matmul(s_ps, qT_sb[:, 0, :], kT_sb[:, 0, :],
                                     start=True, stop=False)
                    nc.tensor.matmul(s_ps, qT_sb[:, 1, :], kT_sb[:, 1, :],
                                     start=False, stop=False)
                    nc.tensor.matmul(s_ps, identr, mask_sb[:, t, :],
                                     start=False, stop=True)

                    pt_sb = ppool.tile([128, NP], F32, tag="pt")
                    se = stpool.tile([128, 1], F32, tag="se")
                    if t == 0:
                        # rows < 31 are fully masked; max-subtraction turns the
                        # all-(-1e30) rows into exp(0)=1 -> uniform p = 1/255
                        mx = stpool.tile([128, 1], F32, tag="mx")
                        nc.vector.reduce_max(mx, s_ps[:, 0:N], axis=AX)
                        bias = stpool.tile([128, 1], F32, tag="bias")
                        nc.scalar.mul(bias, mx, -SCALE)
                        nc.scalar.activation(pt_sb[:, 0:N], s_ps[:, 0:N], EXP,
                                             bias=bias, scale=SCALE, accum_out=se)
                    else:
                        nc.scalar.activation(pt_sb[:, 0:N], s_ps[:, 0:N], EXP,
                                             bias=0.0, scale=SCALE, accum_out=se)

                    rs = stpool.tile([128, 1], F32, tag="rs")
                    nc.vector.reciprocal(rs, se)
                    pn_sb = ppool.tile([128, NP], F32, tag="pn")
                    nc.vector.tensor_scalar_mul(pn_sb[:, 0:N], pt_sb[:, 0:N], rs)

                    nw = N if t == 0 else nhi  # t=0 rows<31 are uniform 1/255
                    nc.sync.dma_start(out=pd[u, m0:m0 + 128, 0:nw],
                                      in_=pn_sb[:, 0:nw])

                    pT_ps = ps_pt.tile([128, NP], F32, tag="pT")
                    nc.tensor.transpose(pT_ps[:, 0:128], pn_sb[:, 0:128], ident32)
                    if nchunks == 2:
                        nc.tensor.transpose(pT_ps[0:N - 128, 128:256],
                                            pn_sb[:, 128:N], ident32)
                    pT_sb = ppool.tile([128, NP], F32, tag="pTs")
                    nc.vector.tensor_copy(pT_sb[:, 0:128 * nchunks],
                                          pT_ps[:, 0:128 * nchunks])

                    o_ps = ps_o.tile([128, VD], F32, tag="o")
                    nc.tensor.matmul(o_ps, pT_sb[:, 0:128], v_sb[:, 0, :],
                                     start=True, stop=(nchunks == 1))
                    if nchunks == 2:
                        nc.tensor.matmul(o_ps, pT_sb[0:N - 128, 128:256],
                                         v_sb[0:127, 1, :],
                                         start=False, stop=True)
                    o_sb = ppool.tile([128, VD], F32, tag="ob")
                    nc.scalar.copy(o_sb, o_ps)
                    if t == 0:
                        nc.vector.memset(o_sb[0:KSZ - 1, :], 0.0)
                    nc.sync.dma_start(out=od[u, m0:m0 + 128, :], in_=o_sb)

    nc.compile()
    return nc


def _host_mask() -> np.ndarray:
    i = np.arange(M, dtype=np.int64).reshape(MT, 128)[:, :, None]
    j = np.arange(NP, dtype=np.int64)[None, None, :]
    return np.where(i < j * STR + KSZ - 1, np.float32(MASKVAL),
                    np.float32(0.0)).astype(np.float32)


def kernel(q, k, v, real_length, kernel_size, stride):
    q = np.asarray(q, dtype=np.float32)
    k = np.asarray(k, dtype=np.float32)
    v = np.asarray(v, dtype=np.float32)
    assert q.shape == (B, M, QH, D) and k.shape == (B, N, QH, D)
    assert int(real_length) == M and int(kernel_size) == KSZ and int(stride) == STR

    if "nc" not in _cache:
        _cache["nc"] = _build()
    nc = _cache["nc"]

    # [b,h]-major shards, contraction-dim-major for q/k
    qTh = np.ascontiguousarray(
        q.transpose(0, 2, 3, 1).reshape(B * QH, D, M))            # [32, 192, 4096]
    kTh = np.zeros((B * QH, D, NP), dtype=np.float32)
    kTh[:, :, 0:N] = k.transpose(0, 2, 3, 1).reshape(B * QH, D, N)
    vh = np.ascontiguousarray(
        v.transpose(0, 2, 1, 3).reshape(B * QH, N, VD))           # [32, 255, 128]
    mask = _host_mask()
    ident = np.eye(128, dtype=np.float32)

    in_maps = []
    for c in range(NCORES):
        sl = slice(c * U, (c + 1) * U)
        in_maps.append({
            "qT": np.ascontiguousarray(qTh[sl]),
            "kT": np.ascontiguousarray(kTh[sl]),
            "v": np.ascontiguousarray(vh[sl]),
            "mask": mask,
            "identr": ident,
            "ident32": ident,
        })

    res = run_bass_kernel_spmd(nc, in_maps, list(range(NCORES)))

    o_sh = np.stack([res.results[c]["o"] for c in range(NCORES)])  # [8, 4, M, VD]
    p_sh = np.stack([res.results[c]["p"] for c in range(NCORES)])  # [8, 4, M, N]
    o = o_sh.reshape(B, QH, M, VD).transpose(0, 2, 1, 3)           # [b, m, h, vd]
    p = p_sh.reshape(B, QH, M, N)                                  # [b, h, m, n]
    return (np.ascontiguousarray(o), np.ascontiguousarray(p))
